# revision 3
# baseline (speedup 1.0000x reference)
"""Trainium2 Bass kernel for nn_CompletenessLoss (OHEM hinge loss with top-k).

Self-contained: accepts FULL inputs, shards over 8 NeuronCores internally
(data-parallel over the group dimension), returns the full scalar output.

Math (reference):
  scores[i]  = pred[i, labels[i]-1]
  groups of 64 rows: first 8 are "positive", last 56 are "negative"
  pos_ls = sum over all positive rows of relu(1 - s)
  neg_ls = sum over groups of (sum of top-9 of relu(1 + s) over 56 negatives)
  out    = (pos_ls + neg_ls) / (num_pos + int(num_neg * 0.17))

V3 strategy (per core, 32768 rows = 128 partitions x 256 rows):
  ALL label gathers run on GPSIMD ap_gather (measured ~2 cyc/idx), fed by a
  16-row-per-partition DMA pipeline of pred in bf16.  ap_gather shares one
  index list per 16-partition core, so the host packs position 16j+q with
  the index of partition (16g+q)'s row j; each partition's wanted PAIR
  (d=2, 32-bit blocks) lands on the "diagonal" slot 16j+(p%16).
  Extraction per 64-row block on DVE: multiply by a static per-partition
  Q-mask (zeroes the 15 wrong slots, keeps the pair), 4 pairwise folds
  (exact: one nonzero pair among zeros), a parity mask picks the even/odd
  element, and a 2-wide reduce writes f32 scores.  Phase 2 (hinge + top-9
  via max8/match_replace) runs per group right behind its block.
  HBM traffic/core: 13.1 MB pred + ~0.7 MB metadata (vs 23 MB in V2).
"""

import numpy as np

# Problem geometry (hardcoded per the harness contract).
N_FULL = 262144
D = 200                      # pred_dim
GS = 64                      # sample_group_size
SS = 8                       # sample_split (positives per group)
OHEM_RATIO = 0.17
KEEP = int((GS - SS) * OHEM_RATIO)   # 9 hardest negatives kept per group

N_CORES = 8
ROWS = N_FULL // N_CORES     # 32768 rows per core
P = 128                      # SBUF partitions
NTILES = ROWS // P           # 256 rows per partition = 4 groups
CHUNK = 64                   # rows-per-partition per DMA/gather chunk
BLK = 64                     # rows-per-partition per extraction block (=GS)
HD = D // 2                  # 100 pair-blocks per row

_compiled = None             # cached program so repeat calls skip rebuild


def build_nc():
    """Build the per-core Bass program (SPMD across the 8 cores)."""
    import concourse.bacc as bacc
    import concourse.tile as tile
    from concourse import mybir

    f32 = mybir.dt.float32
    bf16 = mybir.dt.bfloat16
    i16 = mybir.dt.int16

    nc = bacc.Bacc("TRN2", target_bir_lowering=False, debug=False,
                   num_devices=N_CORES)
    pred_t = nc.dram_tensor("pred", [ROWS, D], bf16, kind="ExternalInput")
    # idx[p, t] = (t%CHUNK)*HD + (lab[p,t]-1)>>1, int16 gather block index
    idx_t = nc.dram_tensor("idx", [P, NTILES], i16, kind="ExternalInput")
    # par[p, 2t+e] = (e == (lab-1)&1), bf16 parity-pair mask
    par_t = nc.dram_tensor("par", [P, NTILES * 2], bf16, kind="ExternalInput")
    # qrep[p, 32j + 2q+e] = (q == p%16), j < BLK: static slot mask
    qrep_t = nc.dram_tensor("qrep", [P, BLK * 32], bf16, kind="ExternalInput")
    out_t = nc.dram_tensor("partial", [P, 2], f32, kind="ExternalOutput")

    with tile.TileContext(nc) as tc:
        _body(tc, pred_t.ap(), idx_t.ap(), par_t.ap(), qrep_t.ap(),
              out_t.ap())
    nc.compile()
    return nc


def _body(tc, pred, idx, par, qrep, out):
    from concourse import mybir
    from contextlib import ExitStack

    nc = tc.nc
    f32 = mybir.dt.float32
    bf16 = mybir.dt.bfloat16
    i16 = mybir.dt.int16
    AX = mybir.AxisListType
    OP = mybir.AluOpType
    AF = mybir.ActivationFunctionType

    with ExitStack() as ctx:
        singles = ctx.enter_context(tc.tile_pool(name="singles", bufs=1))
        ph2 = ctx.enter_context(tc.tile_pool(name="ph2", bufs=2))

        # --- one-time inputs (small, on the scalar queue) ---
        idxs = singles.tile([P, NTILES], i16)
        nc.scalar.dma_start(out=idxs, in_=idx)
        pars = singles.tile([P, NTILES, 2], bf16)
        nc.scalar.dma_start(
            out=pars.rearrange("p t e -> p (t e)"), in_=par)
        qmask = singles.tile([P, BLK * 32], bf16)
        nc.scalar.dma_start(out=qmask, in_=qrep)

        pred_sb = singles.tile([P, NTILES, D], bf16)
        out2 = singles.tile([P, NTILES * 16, 2], bf16)
        scores = singles.tile([P, NTILES], f32)

        # --- warm-up: pay one-time engine costs before the first chunk ---
        # GPSIMD: ap_gather ucode IRAM load (~6us) via a tiny dummy gather.
        wz_idx = singles.tile([P, 16], i16)
        nc.gpsimd.memset(wz_idx, 0)
        wdat = singles.tile([P, 16, 2], bf16)
        nc.gpsimd.memset(wdat.rearrange("p a b -> p (a b)"), 0)
        warm3 = singles.tile([P, 16, 2], bf16)
        nc.gpsimd.ap_gather(out_ap=warm3, in_ap=wdat,
                            idxs_ap=wz_idx[:, 0:1],
                            channels=P, num_elems=16, d=2, num_idxs=16)
        # DVE: first-op dispatch warm.
        wv = singles.tile([P, 2], f32)
        nc.vector.memset(wv, 0.0)
        nc.vector.tensor_scalar(out=wv, in0=wv, scalar1=0.0, scalar2=1.0,
                                op0=OP.mult, op1=OP.mult)
        # Scalar: Relu activation-table load.
        wa = singles.tile([P, 2], f32)
        nc.scalar.activation(out=wa, in_=wv, func=AF.Relu,
                             bias=1.0, scale=-1.0)

        # --- accumulators for phase 2 ---
        gpp = NTILES // GS
        pp = singles.tile([P, gpp], f32)             # pos sums per group
        negacc = singles.tile([P, 2 * gpp], f32)     # top8-sum & 9th cols

        # --- pipeline: DMA chunk -> gather chunk -> (per BLK) extract+phase2
        pred_v = pred.rearrange("(p t) j -> p t j", p=P)
        nchunks = NTILES // CHUNK
        for ci in range(nchunks):
            tb = ci * CHUNK
            nc.sync.dma_start(out=pred_sb[:, tb:tb + CHUNK, :],
                              in_=pred_v[:, tb:tb + CHUNK, :])
            nc.gpsimd.ap_gather(
                out_ap=out2[:, tb * 16:(tb + CHUNK) * 16, :],
                in_ap=pred_sb[:, tb:tb + CHUNK, :].rearrange(
                    "p t (a b) -> p (t a) b", b=2),
                idxs_ap=idxs[:, tb:tb + CHUNK],
                channels=P, num_elems=CHUNK * HD, d=2,
                num_idxs=CHUNK * 16)

            if (tb + CHUNK) % BLK != 0:
                continue

            # --- extraction for the finished 64-row block ---
            bb = (tb + CHUNK) - BLK          # block start row
            g = bb // GS                     # group index (BLK == GS)
            o2 = out2[:, bb * 16:(bb + BLK) * 16, :]
            nc.vector.tensor_tensor(
                out=o2.rearrange("p a b -> p (a b)"),
                in0=o2.rearrange("p a b -> p (a b)"),
                in1=qmask, op=OP.mult)
            o3 = o2.rearrange("p (t a) b -> p t (a b)", a=16)  # [P, 64, 32]
            for half in (16, 8, 4, 2):
                nc.vector.tensor_tensor(
                    out=o3[:, :, 0:half], in0=o3[:, :, 0:half],
                    in1=o3[:, :, half:2 * half], op=OP.add)
            nc.vector.tensor_tensor(
                out=o3[:, :, 0:2], in0=o3[:, :, 0:2],
                in1=pars[:, bb:bb + BLK, :], op=OP.mult)
            nc.vector.tensor_reduce(
                out=scores[:, bb:bb + BLK], in_=o3[:, :, 0:2],
                axis=AX.X, op=OP.add)

            # --- phase 2 for this group: hinge + top-9 ---
            stg = scores[:, g * GS:(g + 1) * GS]
            ptmp = ph2.tile([P, SS], f32, tag="ptmp")
            nc.scalar.activation(
                out=ptmp, in_=stg[:, 0:SS], func=AF.Relu,
                bias=1.0, scale=-1.0, accum_out=pp[:, g:g + 1])
            nl = ph2.tile([P, GS - SS], f32, tag="nl")
            nc.scalar.activation(
                out=nl, in_=stg[:, SS:GS],
                func=AF.Relu, bias=1.0, scale=1.0)
            m8 = ph2.tile([P, 8], f32, tag="m8")
            nc.vector.max(out=m8, in_=nl)
            nc.vector.match_replace(
                out=nl, in_to_replace=m8, in_values=nl, imm_value=-1.0)
            nc.vector.tensor_reduce(
                out=negacc[:, 2 * g:2 * g + 1], in_=m8, axis=AX.X, op=OP.add)
            nc.vector.tensor_reduce(
                out=negacc[:, 2 * g + 1:2 * g + 2], in_=nl, axis=AX.X,
                op=OP.max)

        # --- final per-partition reduction -> [P, 2] ---
        res = singles.tile([P, 2], f32)
        nc.vector.tensor_reduce(out=res[:, 0:1], in_=pp, axis=AX.X, op=OP.add)
        nc.vector.tensor_reduce(out=res[:, 1:2], in_=negacc, axis=AX.X,
                                op=OP.add)
        nc.sync.dma_start(out=out, in_=res)


def _get_compiled():
    global _compiled
    if _compiled is None:
        _compiled = build_nc()
    return _compiled


def _prep_core_inputs(pred, labels):
    """Split full inputs into per-core input maps."""
    import ml_dtypes
    pred = np.asarray(pred).astype(ml_dtypes.bfloat16)
    lab = np.asarray(labels).astype(np.int64)
    jloc = (np.arange(NTILES, dtype=np.int64) % CHUNK)[None, :]  # [1, NT]
    # static slot mask: qrep[p, 32j + 2q+e] = (q == p%16)
    qsel = (np.arange(P, dtype=np.int64) % 16)                   # [P]
    qe = np.arange(32, dtype=np.int64) // 2                      # [32] -> q
    qrep = (qe[None, :] == qsel[:, None]).astype(ml_dtypes.bfloat16)
    qrep = np.ascontiguousarray(np.tile(qrep, (1, BLK)))         # [P, BLK*32]
    in_maps = []
    for c in range(N_CORES):
        sl = slice(c * ROWS, (c + 1) * ROWS)
        lab_sh = (lab[sl] - 1).reshape(P, NTILES)                # int64
        idxs = (jloc * HD + (lab_sh >> 1)).astype(np.int16)
        e = (lab_sh & 1)                                         # [P, NT]
        par = np.zeros((P, NTILES, 2), dtype=ml_dtypes.bfloat16)
        par[:, :, 0] = (e == 0)
        par[:, :, 1] = (e == 1)
        in_maps.append({
            "pred": np.ascontiguousarray(pred[sl]),
            "idx": np.ascontiguousarray(idxs),
            "par": np.ascontiguousarray(par.reshape(P, NTILES * 2)),
            "qrep": qrep,
        })
    return in_maps


def _finalize(results):
    pos = 0.0
    neg = 0.0
    for r in results:
        part = r["partial"].astype(np.float64)
        pos += part[:, 0].sum()
        neg += part[:, 1].sum()
    num_pos = (N_FULL // GS) * SS
    num_neg = N_FULL - num_pos
    denom = float(num_pos + int(num_neg * OHEM_RATIO))
    return np.float32((pos + neg) / denom)


def kernel(pred, labels, sample_split, sample_group_size):
    assert int(sample_split) == SS and int(sample_group_size) == GS
    from concourse.bass_utils import run_bass_kernel_spmd

    nc = _get_compiled()
    in_maps = _prep_core_inputs(pred, labels)
    res = run_bass_kernel_spmd(nc, in_maps, core_ids=list(range(N_CORES)))
    return _finalize(res.results)


# revision 4
# speedup vs baseline: 1.4396x; 1.4396x over previous
"""Trainium2 Bass kernel for nn_CompletenessLoss (OHEM hinge loss with top-k).

Self-contained: accepts FULL inputs, shards over 8 NeuronCores internally
(data-parallel over the group dimension), returns the full scalar output.

Math (reference):
  scores[i]  = pred[i, labels[i]-1]
  groups of 64 rows: first 8 are "positive", last 56 are "negative"
  pos_ls = sum over all positive rows of relu(1 - s)
  neg_ls = sum over groups of (sum of top-9 of relu(1 + s) over 56 negatives)
  out    = (pos_ls + neg_ls) / (num_pos + int(num_neg * 0.17))

V5 strategy (per core, 32768 rows = 128 partitions x 256 rows):
  The label-gather is split across two engines, balanced to their measured
  rates (GPSIMD ap_gather ~28.5ns/idx -> 456ns/row at the 16x per-core
  index amplification; DVE mask-select ~220ns/row; DMA ~390GB/s):
  - lane A, rows t in [0, TA): GPSIMD ap_gather with the diagonal-slot
    trick (shared per-16-partition index lists; each partition's wanted
    bf16 PAIR lands on slot 16j+(p%16)).  DVE extracts via a static Q-mask
    multiply, 4 pairwise folds and a parity-pair select.
  - lane B, rows t in [TA, 256): host-streamed one-hot row masks (bf16,
    400B/row) on a second DMA queue; DVE does mult + fold + reduce.
  Phase 2 (hinge + top-9 via max8/match_replace) runs per 64-row group as
  soon as that group's scores are assembled, ordered so the in-order DVE
  queue never stalls on the slow gather lane while mask-lane work is ready.
"""

import numpy as np

# Problem geometry (hardcoded per the harness contract).
N_FULL = 262144
D = 200                      # pred_dim
GS = 64                      # sample_group_size
SS = 8                       # sample_split (positives per group)
OHEM_RATIO = 0.17
KEEP = int((GS - SS) * OHEM_RATIO)   # 9 hardest negatives kept per group

N_CORES = 8
ROWS = N_FULL // N_CORES     # 32768 rows per core
P = 128                      # SBUF partitions
NTILES = ROWS // P           # 256 rows per partition = 4 groups
HD = D // 2                  # 100 pair-blocks per row

CHA = 28                     # lane-A rows per gather chunk
NCA = 4                      # lane-A chunks
TA = CHA * NCA               # 112 rows on the gather lane
XB = 56                      # lane-A extraction block (2 chunks)
CHB = 16                     # lane-B rows per mask chunk
NB = (NTILES - TA) // CHB    # 9 mask chunks (144 rows)

_compiled = None             # cached program so repeat calls skip rebuild


def build_nc():
    """Build the per-core Bass program (SPMD across the 8 cores)."""
    import concourse.bacc as bacc
    import concourse.tile as tile
    from concourse import mybir

    f32 = mybir.dt.float32
    bf16 = mybir.dt.bfloat16
    i16 = mybir.dt.int16

    nc = bacc.Bacc("TRN2", target_bir_lowering=False, debug=False,
                   num_devices=N_CORES)
    pred_t = nc.dram_tensor("pred", [ROWS, D], bf16, kind="ExternalInput")
    # idx[p, t] = (t%CHA)*HD + (lab[p,t]-1)>>1 for lane-A rows t in [0, TA)
    idx_t = nc.dram_tensor("idx", [P, TA], i16, kind="ExternalInput")
    # par[p, 2t+e] = (e == (lab-1)&1) for lane-A rows, bf16
    par_t = nc.dram_tensor("par", [P, TA * 2], bf16, kind="ExternalInput")
    # qrep[p, 32j + 2q+e] = (q == p%16), j < XB: static slot mask
    qrep_t = nc.dram_tensor("qrep", [P, XB * 32], bf16, kind="ExternalInput")
    # smask: one-hot row masks for lane-B rows, streamed per chunk
    smask_t = nc.dram_tensor("smask", [P, (NTILES - TA) * D], bf16,
                             kind="ExternalInput")
    out_t = nc.dram_tensor("partial", [P, 2], f32, kind="ExternalOutput")

    with tile.TileContext(nc) as tc:
        _body(tc, pred_t.ap(), idx_t.ap(), par_t.ap(), qrep_t.ap(),
              smask_t.ap(), out_t.ap())
    nc.compile()
    return nc


def _body(tc, pred, idx, par, qrep, smask, out):
    from concourse import mybir
    from contextlib import ExitStack

    nc = tc.nc
    f32 = mybir.dt.float32
    bf16 = mybir.dt.bfloat16
    i16 = mybir.dt.int16
    AX = mybir.AxisListType
    OP = mybir.AluOpType
    AF = mybir.ActivationFunctionType

    with ExitStack() as ctx:
        singles = ctx.enter_context(tc.tile_pool(name="singles", bufs=1))
        ph2 = ctx.enter_context(tc.tile_pool(name="ph2", bufs=2))
        scr = ctx.enter_context(tc.tile_pool(name="scr", bufs=4))

        # --- one-time inputs (small, on the scalar queue) ---
        idxs = singles.tile([P, TA], i16)
        nc.scalar.dma_start(out=idxs, in_=idx)
        pars = singles.tile([P, TA, 2], bf16)
        nc.scalar.dma_start(out=pars.rearrange("p t e -> p (t e)"), in_=par)
        qmask = singles.tile([P, XB * 32], bf16)
        nc.scalar.dma_start(out=qmask, in_=qrep)

        pred_sb = singles.tile([P, NTILES, D], bf16)
        out2 = singles.tile([P, TA * 16, 2], bf16)
        scores = singles.tile([P, NTILES], f32)

        # --- warm-up: pay one-time engine costs before the first chunk ---
        wz_idx = singles.tile([P, 16], i16)
        nc.gpsimd.memset(wz_idx, 0)
        wdat = singles.tile([P, 16, 2], bf16)
        nc.gpsimd.memset(wdat.rearrange("p a b -> p (a b)"), 0)
        warm3 = singles.tile([P, 16, 2], bf16)
        nc.gpsimd.ap_gather(out_ap=warm3, in_ap=wdat,
                            idxs_ap=wz_idx[:, 0:1],
                            channels=P, num_elems=16, d=2, num_idxs=16)
        wv = singles.tile([P, 2], f32)
        nc.vector.memset(wv, 0.0)
        nc.vector.tensor_scalar(out=wv, in0=wv, scalar1=0.0, scalar2=1.0,
                                op0=OP.mult, op1=OP.mult)
        wa = singles.tile([P, 2], f32)
        nc.scalar.activation(out=wa, in_=wv, func=AF.Relu,
                             bias=1.0, scale=-1.0)

        # --- accumulators for phase 2 ---
        gpp = NTILES // GS
        pp = singles.tile([P, gpp], f32)             # pos sums per group
        negacc = singles.tile([P, 2 * gpp], f32)     # top8-sum & 9th cols

        pred_v = pred.rearrange("(p t) j -> p t j", p=P)
        sm_v = smask.rearrange("p (t j) -> p t j", j=D)

        # --- DMA issue: lane-A chunks first (gather is the long pole),
        # lane-B pred chunks follow on the same queue; masks on scalar.
        for ci in range(NCA):
            tb = ci * CHA
            nc.sync.dma_start(out=pred_sb[:, tb:tb + CHA, :],
                              in_=pred_v[:, tb:tb + CHA, :])
            nc.gpsimd.ap_gather(
                out_ap=out2[:, tb * 16:(tb + CHA) * 16, :],
                in_ap=pred_sb[:, tb:tb + CHA, :].rearrange(
                    "p t (a b) -> p (t a) b", b=2),
                idxs_ap=idxs[:, tb:tb + CHA],
                channels=P, num_elems=CHA * HD, d=2,
                num_idxs=CHA * 16)

        # lane-B: stream pred + mask chunks; DVE mult+fold+reduce each.
        for bi in range(NB):
            tb = TA + bi * CHB
            nc.sync.dma_start(out=pred_sb[:, tb:tb + CHB, :],
                              in_=pred_v[:, tb:tb + CHB, :])
            mk = scr.tile([P, CHB, D], bf16, tag="mk")
            nc.scalar.dma_start(out=mk,
                                in_=sm_v[:, tb - TA:tb - TA + CHB, :])
            pr = pred_sb[:, tb:tb + CHB, :]
            nc.vector.tensor_tensor(out=pr, in0=pr, in1=mk, op=OP.mult)
            # one nonzero per row: bf16 pairwise folds are exact & 2x
            nc.vector.tensor_tensor(
                out=pr[:, :, 0:D // 2], in0=pr[:, :, 0:D // 2],
                in1=pr[:, :, D // 2:D], op=OP.add)
            nc.vector.tensor_tensor(
                out=pr[:, :, 0:D // 4], in0=pr[:, :, 0:D // 4],
                in1=pr[:, :, D // 4:D // 2], op=OP.add)
            nc.vector.tensor_reduce(
                out=scores[:, tb:tb + CHB], in_=pr[:, :, 0:D // 4],
                axis=AX.X, op=OP.add)

        # lane-A extraction: two 56-row blocks (after gathers 0-1 and 2-3)
        for xb in range(TA // XB):
            bb = xb * XB
            o2 = out2[:, bb * 16:(bb + XB) * 16, :]
            nc.vector.tensor_tensor(
                out=o2.rearrange("p a b -> p (a b)"),
                in0=o2.rearrange("p a b -> p (a b)"),
                in1=qmask, op=OP.mult)
            o3 = o2.rearrange("p (t a) b -> p t (a b)", a=16)  # [P,56,32]
            for half in (16, 8, 4, 2):
                nc.vector.tensor_tensor(
                    out=o3[:, :, 0:half], in0=o3[:, :, 0:half],
                    in1=o3[:, :, half:2 * half], op=OP.add)
            nc.vector.tensor_tensor(
                out=o3[:, :, 0:2], in0=o3[:, :, 0:2],
                in1=pars[:, bb:bb + XB, :], op=OP.mult)
            nc.vector.tensor_reduce(
                out=scores[:, bb:bb + XB], in_=o3[:, :, 0:2],
                axis=AX.X, op=OP.add)

        # --- phase 2 per group: hinge + top-9 ---
        # g0 needs lane-A blocks (t<64); g1 needs both; g2,g3 lane-B only.
        for g in (2, 3, 0, 1):
            stg = scores[:, g * GS:(g + 1) * GS]
            ptmp = ph2.tile([P, SS], f32, tag="ptmp")
            nc.scalar.activation(
                out=ptmp, in_=stg[:, 0:SS], func=AF.Relu,
                bias=1.0, scale=-1.0, accum_out=pp[:, g:g + 1])
            nl = ph2.tile([P, GS - SS], f32, tag="nl")
            nc.scalar.activation(
                out=nl, in_=stg[:, SS:GS],
                func=AF.Relu, bias=1.0, scale=1.0)
            m8 = ph2.tile([P, 8], f32, tag="m8")
            nc.vector.max(out=m8, in_=nl)
            nc.vector.match_replace(
                out=nl, in_to_replace=m8, in_values=nl, imm_value=-1.0)
            nc.vector.tensor_reduce(
                out=negacc[:, 2 * g:2 * g + 1], in_=m8, axis=AX.X, op=OP.add)
            nc.vector.tensor_reduce(
                out=negacc[:, 2 * g + 1:2 * g + 2], in_=nl, axis=AX.X,
                op=OP.max)

        # --- final per-partition reduction -> [P, 2] ---
        res = singles.tile([P, 2], f32)
        nc.vector.tensor_reduce(out=res[:, 0:1], in_=pp, axis=AX.X, op=OP.add)
        nc.vector.tensor_reduce(out=res[:, 1:2], in_=negacc, axis=AX.X,
                                op=OP.add)
        nc.sync.dma_start(out=out, in_=res)


def _get_compiled():
    global _compiled
    if _compiled is None:
        _compiled = build_nc()
    return _compiled


def _prep_core_inputs(pred, labels):
    """Split full inputs into per-core input maps."""
    import ml_dtypes
    pred = np.asarray(pred).astype(ml_dtypes.bfloat16)
    lab = np.asarray(labels).astype(np.int64)
    jloc = (np.arange(TA, dtype=np.int64) % CHA)[None, :]        # [1, TA]
    qsel = (np.arange(P, dtype=np.int64) % 16)                   # [P]
    qe = np.arange(32, dtype=np.int64) // 2                      # [32] -> q
    qrep = (qe[None, :] == qsel[:, None]).astype(ml_dtypes.bfloat16)
    qrep = np.ascontiguousarray(np.tile(qrep, (1, XB)))          # [P, XB*32]
    XS = NTILES - TA
    in_maps = []
    for c in range(N_CORES):
        sl = slice(c * ROWS, (c + 1) * ROWS)
        lab_sh = (lab[sl] - 1).reshape(P, NTILES)                # int64
        la = lab_sh[:, :TA]                                      # [P, TA]
        idxs = (jloc * HD + (la >> 1)).astype(np.int16)
        e = (la & 1)
        par = np.zeros((P, TA, 2), dtype=ml_dtypes.bfloat16)
        par[:, :, 0] = (e == 0)
        par[:, :, 1] = (e == 1)
        ls = lab_sh[:, TA:]                                      # [P, XS]
        smask = np.zeros((P, XS, D), dtype=ml_dtypes.bfloat16)
        smask[np.arange(P)[:, None], np.arange(XS)[None, :], ls] = 1
        in_maps.append({
            "pred": np.ascontiguousarray(pred[sl]),
            "idx": np.ascontiguousarray(idxs),
            "par": np.ascontiguousarray(par.reshape(P, TA * 2)),
            "qrep": qrep,
            "smask": np.ascontiguousarray(smask.reshape(P, XS * D)),
        })
    return in_maps


def _finalize(results):
    pos = 0.0
    neg = 0.0
    for r in results:
        part = r["partial"].astype(np.float64)
        pos += part[:, 0].sum()
        neg += part[:, 1].sum()
    num_pos = (N_FULL // GS) * SS
    num_neg = N_FULL - num_pos
    denom = float(num_pos + int(num_neg * OHEM_RATIO))
    return np.float32((pos + neg) / denom)


def kernel(pred, labels, sample_split, sample_group_size):
    assert int(sample_split) == SS and int(sample_group_size) == GS
    from concourse.bass_utils import run_bass_kernel_spmd

    nc = _get_compiled()
    in_maps = _prep_core_inputs(pred, labels)
    res = run_bass_kernel_spmd(nc, in_maps, core_ids=list(range(N_CORES)))
    return _finalize(res.results)


# revision 8
# speedup vs baseline: 1.4693x; 1.0206x over previous
"""Trainium2 Bass kernel for nn_CompletenessLoss (OHEM hinge loss with top-k).

Self-contained: accepts FULL inputs, shards over 8 NeuronCores internally
(data-parallel over the group dimension), returns the full scalar output.

Math (reference):
  scores[i]  = pred[i, labels[i]-1]
  groups of 64 rows: first 8 are "positive", last 56 are "negative"
  pos_ls = sum over all positive rows of relu(1 - s)
  neg_ls = sum over groups of (sum of top-9 of relu(1 + s) over 56 negatives)
  out    = (pos_ls + neg_ls) / (num_pos + int(num_neg * 0.17))

V5 strategy (per core, 32768 rows = 128 partitions x 256 rows):
  The label-gather is split across two engines, balanced to their measured
  rates (GPSIMD ap_gather ~28.5ns/idx -> 456ns/row at the 16x per-core
  index amplification; DVE mask-select ~220ns/row; DMA ~390GB/s):
  - lane A, rows t in [0, TA): GPSIMD ap_gather with the diagonal-slot
    trick (shared per-16-partition index lists; each partition's wanted
    bf16 PAIR lands on slot 16j+(p%16)).  DVE extracts via a static Q-mask
    multiply, 4 pairwise folds and a parity-pair select.
  - lane B, rows t in [TA, 256): host-streamed one-hot row masks (bf16,
    400B/row) on a second DMA queue; DVE does mult + fold + reduce.
  Phase 2 (hinge + top-9 via max8/match_replace) runs per 64-row group as
  soon as that group's scores are assembled, ordered so the in-order DVE
  queue never stalls on the slow gather lane while mask-lane work is ready.
"""

import numpy as np

# Problem geometry (hardcoded per the harness contract).
N_FULL = 262144
D = 200                      # pred_dim
GS = 64                      # sample_group_size
SS = 8                       # sample_split (positives per group)
OHEM_RATIO = 0.17
KEEP = int((GS - SS) * OHEM_RATIO)   # 9 hardest negatives kept per group

N_CORES = 8
ROWS = N_FULL // N_CORES     # 32768 rows per core
P = 128                      # SBUF partitions
NTILES = ROWS // P           # 256 rows per partition = 4 groups
HD = D // 2                  # 100 pair-blocks per row

CHA = 16                     # lane-A rows per gather chunk
NCA = 4                      # lane-A chunks
TA = CHA * NCA               # 64 rows on the gather lane
XB = 64                      # lane-A extraction block (4 chunks)
CHB = 16                     # lane-B rows per mask chunk
NB = (NTILES - TA) // CHB    # 12 mask chunks (192 rows)

_compiled = None             # cached program so repeat calls skip rebuild


def build_nc():
    """Build the per-core Bass program (SPMD across the 8 cores)."""
    import concourse.bacc as bacc
    import concourse.tile as tile
    from concourse import mybir

    f32 = mybir.dt.float32
    bf16 = mybir.dt.bfloat16
    i16 = mybir.dt.int16

    nc = bacc.Bacc("TRN2", target_bir_lowering=False, debug=False,
                   num_devices=N_CORES)
    pred_t = nc.dram_tensor("pred", [ROWS, D], bf16, kind="ExternalInput")
    # idx[p, t] = (t%CHA)*HD + (lab[p,t]-1)>>1 for lane-A rows t in [0, TA)
    idx_t = nc.dram_tensor("idx", [P, TA], i16, kind="ExternalInput")
    # par[p, 2t+e] = (e == (lab-1)&1) for lane-A rows, bf16
    par_t = nc.dram_tensor("par", [P, TA * 2], bf16, kind="ExternalInput")
    # qrep[p, 32j + 2q+e] = (q == p%16), j < XB: static slot mask
    qrep_t = nc.dram_tensor("qrep", [P, XB * 32], bf16, kind="ExternalInput")
    # smask: one-hot row masks for lane-B rows, streamed per chunk
    smask_t = nc.dram_tensor("smask", [P, (NTILES - TA) * D], bf16,
                             kind="ExternalInput")
    out_t = nc.dram_tensor("partial", [P, 2], f32, kind="ExternalOutput")

    with tile.TileContext(nc) as tc:
        _body(tc, pred_t.ap(), idx_t.ap(), par_t.ap(), qrep_t.ap(),
              smask_t.ap(), out_t.ap())
    nc.compile()
    return nc


def _body(tc, pred, idx, par, qrep, smask, out):
    from concourse import mybir
    from contextlib import ExitStack

    nc = tc.nc
    f32 = mybir.dt.float32
    bf16 = mybir.dt.bfloat16
    i16 = mybir.dt.int16
    AX = mybir.AxisListType
    OP = mybir.AluOpType
    AF = mybir.ActivationFunctionType

    with ExitStack() as ctx:
        singles = ctx.enter_context(tc.tile_pool(name="singles", bufs=1))
        ph2 = ctx.enter_context(tc.tile_pool(name="ph2", bufs=2))
        scr = ctx.enter_context(tc.tile_pool(name="scr", bufs=4))

        # --- one-time inputs (small, on the scalar queue) ---
        idxs = singles.tile([P, TA], i16)
        nc.scalar.dma_start(out=idxs, in_=idx)
        pars = singles.tile([P, TA, 2], bf16)
        nc.scalar.dma_start(out=pars.rearrange("p t e -> p (t e)"), in_=par)
        qmask = singles.tile([P, XB * 32], bf16)
        nc.scalar.dma_start(out=qmask, in_=qrep)

        pred_sb = singles.tile([P, NTILES, D], bf16)
        out2 = singles.tile([P, TA * 16, 2], bf16)
        scores = singles.tile([P, NTILES], f32)

        # --- warm-up: pay one-time engine costs before the first chunk ---
        wz_idx = singles.tile([P, 16], i16)
        nc.gpsimd.memset(wz_idx, 0)
        wdat = singles.tile([P, 16, 2], bf16)
        nc.gpsimd.memset(wdat.rearrange("p a b -> p (a b)"), 0)
        warm3 = singles.tile([P, 16, 2], bf16)
        nc.gpsimd.ap_gather(out_ap=warm3, in_ap=wdat,
                            idxs_ap=wz_idx[:, 0:1],
                            channels=P, num_elems=16, d=2, num_idxs=16)
        wv = singles.tile([P, 2], f32)
        nc.vector.memset(wv, 0.0)
        nc.vector.tensor_scalar(out=wv, in0=wv, scalar1=0.0, scalar2=1.0,
                                op0=OP.mult, op1=OP.mult)
        wa = singles.tile([P, 2], f32)
        nc.scalar.activation(out=wa, in_=wv, func=AF.Relu,
                             bias=1.0, scale=-1.0)

        # --- accumulators for phase 2 ---
        gpp = NTILES // GS
        pp = singles.tile([P, gpp], f32)             # pos sums per group
        negacc = singles.tile([P, 2 * gpp], f32)     # top8-sum & 9th cols

        pred_v = pred.rearrange("(p t) j -> p t j", p=P)
        sm_v = smask.rearrange("p (t j) -> p t j", j=D)

        # --- DMA issue: lane-A chunks first (gather is the long pole),
        # lane-B pred chunks follow on the same queue; masks on scalar.
        for ci in range(NCA):
            tb = ci * CHA
            nc.sync.dma_start(out=pred_sb[:, tb:tb + CHA, :],
                              in_=pred_v[:, tb:tb + CHA, :])
            nc.gpsimd.ap_gather(
                out_ap=out2[:, tb * 16:(tb + CHA) * 16, :],
                in_ap=pred_sb[:, tb:tb + CHA, :].rearrange(
                    "p t (a b) -> p (t a) b", b=2),
                idxs_ap=idxs[:, tb:tb + CHA],
                channels=P, num_elems=CHA * HD, d=2,
                num_idxs=CHA * 16)

        def phase2(g):
            """hinge + top-9 for group g (reads scores[:, g*GS:(g+1)*GS])."""
            stg = scores[:, g * GS:(g + 1) * GS]
            ptmp = ph2.tile([P, SS], f32, tag="ptmp")
            nc.scalar.activation(
                out=ptmp, in_=stg[:, 0:SS], func=AF.Relu,
                bias=1.0, scale=-1.0, accum_out=pp[:, g:g + 1])
            nl = ph2.tile([P, GS - SS], f32, tag="nl")
            nc.scalar.activation(
                out=nl, in_=stg[:, SS:GS],
                func=AF.Relu, bias=1.0, scale=1.0)
            m8 = ph2.tile([P, 8], f32, tag="m8")
            nc.vector.max(out=m8, in_=nl)
            nc.vector.match_replace(
                out=nl, in_to_replace=m8, in_values=nl, imm_value=-1.0)
            nc.vector.tensor_reduce(
                out=negacc[:, 2 * g:2 * g + 1], in_=m8, axis=AX.X, op=OP.add)
            nc.vector.tensor_reduce(
                out=negacc[:, 2 * g + 1:2 * g + 2], in_=nl, axis=AX.X,
                op=OP.max)

        # lane-B: stream pred + mask chunks; DVE mult+fold+reduce each.
        # Interleave phase 2 for a group as soon as its rows are scored.
        for bi in range(NB):
            tb = TA + bi * CHB
            nc.sync.dma_start(out=pred_sb[:, tb:tb + CHB, :],
                              in_=pred_v[:, tb:tb + CHB, :])
            mk = scr.tile([P, CHB, D], bf16, tag="mk")
            nc.scalar.dma_start(out=mk,
                                in_=sm_v[:, tb - TA:tb - TA + CHB, :])
            pr = pred_sb[:, tb:tb + CHB, :]
            nc.vector.tensor_tensor(out=pr, in0=pr, in1=mk, op=OP.mult)
            # one nonzero per row: bf16 pairwise folds are exact & 2x
            nc.vector.tensor_tensor(
                out=pr[:, :, 0:D // 2], in0=pr[:, :, 0:D // 2],
                in1=pr[:, :, D // 2:D], op=OP.add)
            nc.vector.tensor_tensor(
                out=pr[:, :, 0:D // 4], in0=pr[:, :, 0:D // 4],
                in1=pr[:, :, D // 4:D // 2], op=OP.add)
            nc.vector.tensor_tensor(
                out=pr[:, :, 0:D // 8], in0=pr[:, :, 0:D // 8],
                in1=pr[:, :, D // 8:D // 4], op=OP.add)
            nc.vector.tensor_reduce(
                out=scores[:, tb:tb + CHB], in_=pr[:, :, 0:D // 8],
                axis=AX.X, op=OP.add)
            if (tb + CHB) % GS == 0:
                phase2((tb + CHB) // GS - 1)

        # lane-A extraction: one 64-row block (group 0), after the gathers.
        o2 = out2[:, 0:XB * 16, :]
        nc.vector.tensor_tensor(
            out=o2.rearrange("p a b -> p (a b)"),
            in0=o2.rearrange("p a b -> p (a b)"),
            in1=qmask, op=OP.mult)
        o3 = o2.rearrange("p (t a) b -> p t (a b)", a=16)  # [P,64,32]
        for half in (16, 8, 4, 2):
            nc.vector.tensor_tensor(
                out=o3[:, :, 0:half], in0=o3[:, :, 0:half],
                in1=o3[:, :, half:2 * half], op=OP.add)
        nc.vector.tensor_tensor(
            out=o3[:, :, 0:2], in0=o3[:, :, 0:2],
            in1=pars[:, 0:XB, :], op=OP.mult)
        nc.vector.tensor_reduce(
            out=scores[:, 0:XB], in_=o3[:, :, 0:2],
            axis=AX.X, op=OP.add)
        phase2(0)

        # --- final per-partition reduction -> [P, 2] ---
        res = singles.tile([P, 2], f32)
        nc.vector.tensor_reduce(out=res[:, 0:1], in_=pp, axis=AX.X, op=OP.add)
        nc.vector.tensor_reduce(out=res[:, 1:2], in_=negacc, axis=AX.X,
                                op=OP.add)
        nc.sync.dma_start(out=out, in_=res)


def _get_compiled():
    global _compiled
    if _compiled is None:
        _compiled = build_nc()
    return _compiled


def _prep_core_inputs(pred, labels):
    """Split full inputs into per-core input maps."""
    import ml_dtypes
    pred = np.asarray(pred).astype(ml_dtypes.bfloat16)
    lab = np.asarray(labels).astype(np.int64)
    jloc = (np.arange(TA, dtype=np.int64) % CHA)[None, :]        # [1, TA]
    qsel = (np.arange(P, dtype=np.int64) % 16)                   # [P]
    qe = np.arange(32, dtype=np.int64) // 2                      # [32] -> q
    qrep = (qe[None, :] == qsel[:, None]).astype(ml_dtypes.bfloat16)
    qrep = np.ascontiguousarray(np.tile(qrep, (1, XB)))          # [P, XB*32]
    XS = NTILES - TA
    in_maps = []
    for c in range(N_CORES):
        sl = slice(c * ROWS, (c + 1) * ROWS)
        lab_sh = (lab[sl] - 1).reshape(P, NTILES)                # int64
        la = lab_sh[:, :TA]                                      # [P, TA]
        idxs = (jloc * HD + (la >> 1)).astype(np.int16)
        e = (la & 1)
        par = np.zeros((P, TA, 2), dtype=ml_dtypes.bfloat16)
        par[:, :, 0] = (e == 0)
        par[:, :, 1] = (e == 1)
        ls = lab_sh[:, TA:]                                      # [P, XS]
        smask = np.zeros((P, XS, D), dtype=ml_dtypes.bfloat16)
        smask[np.arange(P)[:, None], np.arange(XS)[None, :], ls] = 1
        in_maps.append({
            "pred": np.ascontiguousarray(pred[sl]),
            "idx": np.ascontiguousarray(idxs),
            "par": np.ascontiguousarray(par.reshape(P, TA * 2)),
            "qrep": qrep,
            "smask": np.ascontiguousarray(smask.reshape(P, XS * D)),
        })
    return in_maps


def _finalize(results):
    pos = 0.0
    neg = 0.0
    for r in results:
        part = r["partial"].astype(np.float64)
        pos += part[:, 0].sum()
        neg += part[:, 1].sum()
    num_pos = (N_FULL // GS) * SS
    num_neg = N_FULL - num_pos
    denom = float(num_pos + int(num_neg * OHEM_RATIO))
    return np.float32((pos + neg) / denom)


def kernel(pred, labels, sample_split, sample_group_size):
    assert int(sample_split) == SS and int(sample_group_size) == GS
    from concourse.bass_utils import run_bass_kernel_spmd

    nc = _get_compiled()
    in_maps = _prep_core_inputs(pred, labels)
    res = run_bass_kernel_spmd(nc, in_maps, core_ids=list(range(N_CORES)))
    return _finalize(res.results)


# revision 18
# speedup vs baseline: 1.5205x; 1.0349x over previous
"""Trainium2 Bass kernel for nn_CompletenessLoss (OHEM hinge loss with top-k).

Self-contained: accepts FULL inputs, shards over 8 NeuronCores internally
(data-parallel over the group dimension), returns the full scalar output.

Math (reference):
  scores[i]  = pred[i, labels[i]-1]
  groups of 64 rows: first 8 are "positive", last 56 are "negative"
  pos_ls = sum over all positive rows of relu(1 - s)
  neg_ls = sum over groups of (sum of top-9 of relu(1 + s) over 56 negatives)
  out    = (pos_ls + neg_ls) / (num_pos + int(num_neg * 0.17))

V2 gather strategy (per core, 32768 rows = 128 partitions x 256 rows):
  The label-indexed gather is split across two engines running in parallel:
  - rows t in [0, XG): GPSIMD ap_gather. Indices are shared per 16-partition
    group, so the host packs index lists where position 16k+q holds the
    index of partition (16g+q)'s row k; each partition's wanted value lands
    on the "diagonal" position 16k+(p%16). A static-per-input mask multiply
    + segmented reduce (DVE, cheap) extracts the diagonal.
  - rows t in [XG, 256): DVE scalar_tensor_tensor (iota==label)*pred with
    fused accumulate (the V1 path).
  Both write into one scores[P, 256] tile; phase 2 (hinge + top-9) as in V1.

  Measured constraints that pin this design (from session traces):
  - ap_gather costs ~28.5ns/index (RD_CMD serialization, ReadOverlap=0),
    independent of instruction granularity -> the gather lane saturates at
    ~80-100 rows within the DMA window.
  - DVE mask-select costs ~270ns/row incl. the one-hot mask DMA (400B/row),
    and GPSIMD shares its SBUF port with DVE (gathers run ~2x slower while
    DVE is busy), so the kernel is jointly DMA- and DVE-bound at ~98us.
"""

import numpy as np

# Problem geometry (hardcoded per the harness contract).
N_FULL = 262144
D = 200                      # pred_dim
GS = 64                      # sample_group_size
SS = 8                       # sample_split (positives per group)
OHEM_RATIO = 0.17
KEEP = int((GS - SS) * OHEM_RATIO)   # 9 hardest negatives kept per group

N_CORES = 8
ROWS = N_FULL // N_CORES     # 32768 rows per core
P = 128                      # SBUF partitions
NTILES = ROWS // P           # 256 rows per partition = 4 groups
CHUNK = 16                   # rows-per-partition per DMA/compute chunk
XG = 80                      # rows per partition gathered on GPSIMD
XS = NTILES - XG             # rows via host one-hot mask + DVE mult/reduce
XD = 0

_compiled = None             # cached program so repeat calls skip rebuild


def build_nc():
    """Build the per-core Bass program (SPMD across the 8 cores)."""
    import concourse.bacc as bacc
    import concourse.tile as tile
    from concourse import mybir

    f32 = mybir.dt.float32
    bf16 = mybir.dt.bfloat16
    i16 = mybir.dt.int16

    nc = bacc.Bacc("TRN2", target_bir_lowering=False, debug=False,
                   num_devices=N_CORES)
    pred_t = nc.dram_tensor("pred", [ROWS, D], bf16, kind="ExternalInput")
    # idx[p, t] = (t%16)*100 + lab>>1 for gpsimd rows t in [0, XG)
    idx_t = nc.dram_tensor("idx", [P, XG], i16, kind="ExternalInput")
    # msk[p, t*32 + q*2 + e] = (q == p%16) & (e == lab%2), bf16
    msk_t = nc.dram_tensor("msk", [P, XG * 32], bf16, kind="ExternalInput")
    # smask: host one-hot mask for the s-lane rows, streamed per chunk
    smask_t = nc.dram_tensor("smask", [P, XS * D], bf16, kind="ExternalInput")
    out_t = nc.dram_tensor("partial", [P, 2], f32, kind="ExternalOutput")

    with tile.TileContext(nc) as tc:
        _body(tc, pred_t.ap(), idx_t.ap(), msk_t.ap(), smask_t.ap(),
              out_t.ap())
    nc.compile()
    return nc


def _body(tc, pred, idx, msk, smask, out):
    from concourse import mybir
    import concourse.bass as bass
    from contextlib import ExitStack

    nc = tc.nc
    f32 = mybir.dt.float32
    bf16 = mybir.dt.bfloat16
    i16 = mybir.dt.int16
    AX = mybir.AxisListType
    OP = mybir.AluOpType
    AF = mybir.ActivationFunctionType

    with ExitStack() as ctx:
        singles = ctx.enter_context(tc.tile_pool(name="singles", bufs=1))
        ph2 = ctx.enter_context(tc.tile_pool(name="ph2", bufs=2))
        scr = ctx.enter_context(tc.tile_pool(name="scr", bufs=4))

        # --- warm-up FIRST, with zero DMA dependencies, so the gather
        # ucode IRAM load (~6us) and engine dispatch are paid by ~15us.
        wz_idx = singles.tile([P, 16], i16)
        nc.gpsimd.memset(wz_idx, 0)
        wdat = singles.tile([P, 16, 2], bf16)
        nc.gpsimd.memset(wdat.rearrange("p a b -> p (a b)"), 0)
        warm3 = singles.tile([P, 16, 2], bf16)
        nc.gpsimd.ap_gather(out_ap=warm3, in_ap=wdat,
                            idxs_ap=wz_idx[:, 0:1],
                            channels=P, num_elems=16, d=2, num_idxs=16)
        wv = singles.tile([P, 2], f32)
        nc.vector.memset(wv, 0.0)
        nc.vector.tensor_scalar(out=wv, in0=wv, scalar1=0.0, scalar2=1.0,
                                op0=OP.mult, op1=OP.mult)
        wa = singles.tile([P, 2], f32)
        nc.scalar.activation(out=wa, in_=wv, func=AF.Relu,
                             bias=1.0, scale=-1.0)

        # --- one-time inputs: idx first (gathers need it early); the
        # extract masks (msks) are DMA'd after the first two smask chunks
        # so lane-B's first chunk lands as early as possible.
        idxs = singles.tile([P, XG], i16)
        nc.scalar.dma_start(out=idxs, in_=idx)
        msks = singles.tile([P, XG, 32], bf16)

        pred_sb = singles.tile([P, NTILES, D], bf16)
        out2 = singles.tile([P, XG * 16, 2], bf16)
        scores = singles.tile([P, NTILES], f32)

        # --- phase 1: stream pred; gather on GPSIMD + DVE in parallel ---
        pred_v = pred.rearrange("(p t) j -> p t j", p=P)
        # gather chunks grow geometrically (amortize ~2.4us/op overhead);
        # DVE chunks stay small for pipelining. DMA order feeds both early.
        gchunks = [(i * CHUNK, CHUNK) for i in range(XG // CHUNK)]
        schunks = [(XG + i * CHUNK, CHUNK) for i in range(XS // CHUNK)]
        dchunks = [(XG + XS + i * CHUNK, CHUNK) for i in range(XD // CHUNK)]
        order = []
        gi, si, di = 0, 0, 0
        while gi < len(gchunks) or si < len(schunks) or di < len(dchunks):
            if gi < len(gchunks):
                order.append(("g", gchunks[gi])); gi += 1
            if si < len(schunks):
                order.append(("s", schunks[si])); si += 1
            if si < len(schunks):
                order.append(("s", schunks[si])); si += 1
            if di < len(dchunks):
                order.append(("d", dchunks[di])); di += 1

        n_s_issued = 0
        for kind, (tb, csz) in order:
            nc.sync.dma_start(out=pred_sb[:, tb:tb + csz, :],
                              in_=pred_v[:, tb:tb + csz, :])
            if kind == "s":
                n_s_issued += 1
                if n_s_issued == 3:
                    nc.scalar.dma_start(out=msks, in_=msk)
            if kind == "g":
                # gpsimd gather: shared indices per 16-partition group
                nc.gpsimd.ap_gather(
                    out_ap=out2[:, tb * 16:(tb + csz) * 16, :],
                    in_ap=pred_sb[:, tb:tb + csz, :].rearrange(
                        "p t (a b) -> p (t a) b", b=2),
                    idxs_ap=idxs[:, tb:tb + csz],
                    channels=P, num_elems=csz * (D // 2), d=2,
                    num_idxs=csz * 16)
            elif kind == "s":
                # host one-hot mask chunk arrives on the scalar DMA queue;
                # DVE does bulk 2x multiply (in place over pred) + seg reduce
                mk = scr.tile([P, CHUNK, D], bf16, tag="mk")
                sm_v = smask.rearrange("p (t j) -> p t j", j=D)
                nc.scalar.dma_start(out=mk[:, 0:csz, :],
                                     in_=sm_v[:, tb - XG:tb - XG + csz, :])
                pr = pred_sb[:, tb:tb + csz, :]
                nc.vector.tensor_tensor(out=pr, in0=pr, in1=mk[:, 0:csz, :],
                                        op=OP.mult)
                # masked rows are one-nonzero-among-zeros: bf16 pairwise
                # fold is exact and runs at 2x; the 1x reduce sees half
                nc.vector.tensor_tensor(
                    out=pr[:, :, 0:D // 2], in0=pr[:, :, 0:D // 2],
                    in1=pr[:, :, D // 2:D], op=OP.add)
                nc.vector.tensor_tensor(
                    out=pr[:, :, 0:D // 4], in0=pr[:, :, 0:D // 4],
                    in1=pr[:, :, D // 4:D // 2], op=OP.add)
                nc.vector.tensor_tensor(
                    out=pr[:, :, 0:D // 8], in0=pr[:, :, 0:D // 8],
                    in1=pr[:, :, D // 8:D // 4], op=OP.add)
                nc.vector.tensor_reduce(
                    out=scores[:, tb:tb + csz], in_=pr[:, :, 0:D // 8],
                    axis=AX.X, op=OP.add)
            else:
                for b in range(csz):
                    t = tb + b
                    nc.vector.scalar_tensor_tensor(
                        out=pred_sb[:, t, :], in0=iota,
                        scalar=labs[:, t:t + 1],
                        in1=pred_sb[:, t, :], op0=OP.is_equal, op1=OP.mult,
                        accum_out=scores[:, t:t + 1])

        # extracts AFTER all stt issues: the Vector queue is in-order, so an
        # extract waiting on a late gather must not block pending stt work.
        for tb, csz in gchunks:
            o2 = out2[:, tb * 16:(tb + csz) * 16, :]
            nc.vector.tensor_tensor(
                out=o2, in0=o2,
                in1=msks[:, tb:tb + csz, :].rearrange(
                    "p t (a b) -> p (t a) b", b=2),
                op=OP.mult)
            o3 = out2[:, tb * 16:(tb + csz) * 16, :].rearrange(
                "p (t a) b -> p t (a b)", a=16)
            nc.vector.tensor_tensor(
                out=o3[:, :, 0:16], in0=o3[:, :, 0:16], in1=o3[:, :, 16:32],
                op=OP.add)
            nc.vector.tensor_reduce(
                out=scores[:, tb:tb + csz], in_=o3[:, :, 0:16],
                axis=AX.X, op=OP.add)

        # --- phase 2: per partition, 4 whole groups along the free axis ---
        gpp = NTILES // GS
        pp = singles.tile([P, gpp], f32)             # pos sums per group
        negacc = singles.tile([P, 2 * gpp], f32)     # top8-sum & 9th cols
        for g in range(gpp):
            stg = scores[:, g * GS:(g + 1) * GS]
            ptmp = ph2.tile([P, SS], f32, tag="ptmp")
            nc.scalar.activation(
                out=ptmp, in_=stg[:, 0:SS], func=AF.Relu,
                bias=1.0, scale=-1.0, accum_out=pp[:, g:g + 1])
            nl = ph2.tile([P, GS - SS], f32, tag="nl")
            nc.scalar.activation(
                out=nl, in_=stg[:, SS:GS],
                func=AF.Relu, bias=1.0, scale=1.0)
            m8 = ph2.tile([P, 8], f32, tag="m8")
            nc.vector.max(out=m8, in_=nl)
            nc.vector.match_replace(
                out=nl, in_to_replace=m8, in_values=nl, imm_value=-1.0)
            s8 = ph2.tile([P, 8], f32, tag="s8")
            nc.scalar.activation(
                out=s8, in_=m8, func=AF.Relu, bias=0.0, scale=1.0,
                accum_out=negacc[:, 2 * g:2 * g + 1])
            nc.vector.tensor_reduce(
                out=negacc[:, 2 * g + 1:2 * g + 2], in_=nl, axis=AX.X,
                op=OP.max)

        # --- final per-partition reduction -> [P, 2] (on Scalar) ---
        res = singles.tile([P, 2], f32)
        fp = ph2.tile([P, gpp], f32, tag="fp")
        nc.scalar.activation(out=fp, in_=pp, func=AF.Relu, bias=0.0,
                             scale=1.0, accum_out=res[:, 0:1])
        fn = ph2.tile([P, 2 * gpp], f32, tag="fn")
        nc.scalar.activation(out=fn, in_=negacc, func=AF.Relu, bias=0.0,
                             scale=1.0, accum_out=res[:, 1:2])
        nc.sync.dma_start(out=out, in_=res)


def _get_compiled():
    global _compiled
    if _compiled is None:
        _compiled = build_nc()
    return _compiled


def _prep_core_inputs(pred, labels):
    """Split full inputs into per-core input maps."""
    import ml_dtypes
    pred = np.asarray(pred).astype(ml_dtypes.bfloat16)
    lab = np.asarray(labels).astype(np.int64)
    k16 = (np.arange(XG, dtype=np.int64) % CHUNK)[None, :]      # [1, XG]
    qsel = (np.arange(P, dtype=np.int64) % 16)                  # [P]
    in_maps = []
    for c in range(N_CORES):
        sl = slice(c * ROWS, (c + 1) * ROWS)
        lab_sh = (lab[sl] - 1).reshape(P, NTILES)                # int64
        lg = lab_sh[:, :XG]                                      # [P, XG]
        idxs = (k16 * (D // 2) + (lg >> 1)).astype(np.int16)
        # msk[p, t, q*2+e] = (q == p%16) & (e == lab%2)
        msk = np.zeros((P, XG, 32), dtype=ml_dtypes.bfloat16)
        e = (lg & 1).astype(np.int64)                            # [P, XG]
        pi = np.arange(P)[:, None]
        ti = np.arange(XG)[None, :]
        msk[pi, ti, qsel[:, None] * 2 + e] = 1
        ls = lab_sh[:, XG:]                                      # [P, XS]
        smask = np.zeros((P, XS, D), dtype=ml_dtypes.bfloat16)
        smask[np.arange(P)[:, None], np.arange(XS)[None, :], ls] = 1
        in_maps.append({
            "pred": np.ascontiguousarray(pred[sl]),
            "smask": np.ascontiguousarray(smask.reshape(P, XS * D)),
            "idx": np.ascontiguousarray(idxs),
            "msk": np.ascontiguousarray(msk.reshape(P, XG * 32)),
        })
    return in_maps


def _finalize(results):
    pos = 0.0
    neg = 0.0
    for r in results:
        part = r["partial"].astype(np.float64)
        pos += part[:, 0].sum()
        neg += part[:, 1].sum()
    num_pos = (N_FULL // GS) * SS
    num_neg = N_FULL - num_pos
    denom = float(num_pos + int(num_neg * OHEM_RATIO))
    return np.float32((pos + neg) / denom)


def kernel(pred, labels, sample_split, sample_group_size):
    assert int(sample_split) == SS and int(sample_group_size) == GS
    from concourse.bass_utils import run_bass_kernel_spmd

    nc = _get_compiled()
    in_maps = _prep_core_inputs(pred, labels)
    res = run_bass_kernel_spmd(nc, in_maps, core_ids=list(range(N_CORES)))
    return _finalize(res.results)


# revision 19
# speedup vs baseline: 1.5227x; 1.0015x over previous
"""Trainium2 Bass kernel for nn_CompletenessLoss (OHEM hinge loss with top-k).

Self-contained: accepts FULL inputs, shards over 8 NeuronCores internally
(data-parallel over the group dimension), returns the full scalar output.

Math (reference):
  scores[i]  = pred[i, labels[i]-1]
  groups of 64 rows: first 8 are "positive", last 56 are "negative"
  pos_ls = sum over all positive rows of relu(1 - s)
  neg_ls = sum over groups of (sum of top-9 of relu(1 + s) over 56 negatives)
  out    = (pos_ls + neg_ls) / (num_pos + int(num_neg * 0.17))

V2 gather strategy (per core, 32768 rows = 128 partitions x 256 rows):
  The label-indexed gather is split across two engines running in parallel:
  - rows t in [0, XG): GPSIMD ap_gather. Indices are shared per 16-partition
    group, so the host packs index lists where position 16k+q holds the
    index of partition (16g+q)'s row k; each partition's wanted value lands
    on the "diagonal" position 16k+(p%16). A static-per-input mask multiply
    + segmented reduce (DVE, cheap) extracts the diagonal.
  - rows t in [XG, 256): DVE scalar_tensor_tensor (iota==label)*pred with
    fused accumulate (the V1 path).
  Both write into one scores[P, 256] tile; phase 2 (hinge + top-9) as in V1.

  Measured constraints that pin this design (from session traces):
  - ap_gather costs ~28.5ns/index (RD_CMD serialization, ReadOverlap=0),
    independent of instruction granularity -> the gather lane saturates at
    ~80-100 rows within the DMA window.
  - DVE mask-select costs ~270ns/row incl. the one-hot mask DMA (400B/row),
    and GPSIMD shares its SBUF port with DVE (gathers run ~2x slower while
    DVE is busy), so the kernel is jointly DMA- and DVE-bound at ~98us.
"""

import numpy as np

# Problem geometry (hardcoded per the harness contract).
N_FULL = 262144
D = 200                      # pred_dim
GS = 64                      # sample_group_size
SS = 8                       # sample_split (positives per group)
OHEM_RATIO = 0.17
KEEP = int((GS - SS) * OHEM_RATIO)   # 9 hardest negatives kept per group

N_CORES = 8
ROWS = N_FULL // N_CORES     # 32768 rows per core
P = 128                      # SBUF partitions
NTILES = ROWS // P           # 256 rows per partition = 4 groups
CHUNK = 16                   # rows-per-partition per DMA/compute chunk
XG = 80                      # rows per partition gathered on GPSIMD
XS = NTILES - XG             # rows via host one-hot mask + DVE mult/reduce
XD = 0

_compiled = None             # cached program so repeat calls skip rebuild


def build_nc():
    """Build the per-core Bass program (SPMD across the 8 cores)."""
    import concourse.bacc as bacc
    import concourse.tile as tile
    from concourse import mybir

    f32 = mybir.dt.float32
    bf16 = mybir.dt.bfloat16
    i16 = mybir.dt.int16

    nc = bacc.Bacc("TRN2", target_bir_lowering=False, debug=False,
                   num_devices=N_CORES)
    pred_t = nc.dram_tensor("pred", [ROWS, D], bf16, kind="ExternalInput")
    # idx[p, t] = (t%16)*100 + lab>>1 for gpsimd rows t in [0, XG)
    idx_t = nc.dram_tensor("idx", [P, XG], i16, kind="ExternalInput")
    # msk[p, t*32 + q*2 + e] = (q == p%16) & (e == lab%2), bf16
    msk_t = nc.dram_tensor("msk", [P, XG * 32], bf16, kind="ExternalInput")
    # smask: host one-hot mask for the s-lane rows, streamed per chunk
    smask_t = nc.dram_tensor("smask", [P, XS * D], bf16, kind="ExternalInput")
    out_t = nc.dram_tensor("partial", [P, 2], f32, kind="ExternalOutput")

    with tile.TileContext(nc) as tc:
        _body(tc, pred_t.ap(), idx_t.ap(), msk_t.ap(), smask_t.ap(),
              out_t.ap())
    nc.compile()
    return nc


def _body(tc, pred, idx, msk, smask, out):
    from concourse import mybir
    import concourse.bass as bass
    from contextlib import ExitStack

    nc = tc.nc
    f32 = mybir.dt.float32
    bf16 = mybir.dt.bfloat16
    i16 = mybir.dt.int16
    AX = mybir.AxisListType
    OP = mybir.AluOpType
    AF = mybir.ActivationFunctionType

    with ExitStack() as ctx:
        singles = ctx.enter_context(tc.tile_pool(name="singles", bufs=1))
        ph2 = ctx.enter_context(tc.tile_pool(name="ph2", bufs=2))
        scr = ctx.enter_context(tc.tile_pool(name="scr", bufs=4))

        # --- warm-up FIRST, with zero DMA dependencies, so the gather
        # ucode IRAM load (~6us) and engine dispatch are paid by ~15us.
        wz_idx = singles.tile([P, 16], i16)
        nc.gpsimd.memset(wz_idx, 0)
        wdat = singles.tile([P, 16, 2], bf16)
        nc.gpsimd.memset(wdat.rearrange("p a b -> p (a b)"), 0)
        warm3 = singles.tile([P, 16, 2], bf16)
        nc.gpsimd.ap_gather(out_ap=warm3, in_ap=wdat,
                            idxs_ap=wz_idx[:, 0:1],
                            channels=P, num_elems=16, d=2, num_idxs=16)
        wv = singles.tile([P, 2], f32)
        nc.vector.memset(wv, 0.0)
        nc.vector.tensor_scalar(out=wv, in0=wv, scalar1=0.0, scalar2=1.0,
                                op0=OP.mult, op1=OP.mult)
        wa = singles.tile([P, 2], f32)
        nc.scalar.activation(out=wa, in_=wv, func=AF.Relu,
                             bias=1.0, scale=-1.0)

        # --- one-time inputs: idx first (gathers need it early); the
        # extract masks (msks) are DMA'd after the first two smask chunks
        # so lane-B's first chunk lands as early as possible.
        idxs = singles.tile([P, XG], i16)
        nc.scalar.dma_start(out=idxs, in_=idx)
        msks = singles.tile([P, XG, 32], bf16)

        pred_sb = singles.tile([P, NTILES, D], bf16)
        out2 = singles.tile([P, XG * 16, 2], bf16)
        scores = singles.tile([P, NTILES], f32)

        # --- phase 1: stream pred; gather on GPSIMD + DVE in parallel ---
        pred_v = pred.rearrange("(p t) j -> p t j", p=P)
        # gather chunks grow geometrically (amortize ~2.4us/op overhead);
        # DVE chunks stay small for pipelining. DMA order feeds both early.
        gchunks = [(i * CHUNK, CHUNK) for i in range(XG // CHUNK)]
        schunks = [(XG + i * CHUNK, CHUNK) for i in range(XS // CHUNK)]
        dchunks = [(XG + XS + i * CHUNK, CHUNK) for i in range(XD // CHUNK)]
        order = []
        gi, si, di = 0, 0, 0
        # 1:1 g/s interleave: the gather lane is contention-paced at
        # ~13us/chunk, so its pred chunks must all land by ~30us or the
        # chain goes data-starved at the tail (measured: g4 at 84us with
        # the old 1g:2s order).
        while gi < len(gchunks) or si < len(schunks) or di < len(dchunks):
            if gi < len(gchunks):
                order.append(("g", gchunks[gi])); gi += 1
            if si < len(schunks):
                order.append(("s", schunks[si])); si += 1
            if di < len(dchunks):
                order.append(("d", dchunks[di])); di += 1

        n_s_issued = 0
        for kind, (tb, csz) in order:
            nc.sync.dma_start(out=pred_sb[:, tb:tb + csz, :],
                              in_=pred_v[:, tb:tb + csz, :])
            if kind == "s":
                n_s_issued += 1
                if n_s_issued == 3:
                    nc.scalar.dma_start(out=msks, in_=msk)
            if kind == "g":
                # gpsimd gather: shared indices per 16-partition group
                nc.gpsimd.ap_gather(
                    out_ap=out2[:, tb * 16:(tb + csz) * 16, :],
                    in_ap=pred_sb[:, tb:tb + csz, :].rearrange(
                        "p t (a b) -> p (t a) b", b=2),
                    idxs_ap=idxs[:, tb:tb + csz],
                    channels=P, num_elems=csz * (D // 2), d=2,
                    num_idxs=csz * 16)
            elif kind == "s":
                # host one-hot mask chunk arrives on the scalar DMA queue;
                # DVE does bulk 2x multiply (in place over pred) + seg reduce
                mk = scr.tile([P, CHUNK, D], bf16, tag="mk")
                sm_v = smask.rearrange("p (t j) -> p t j", j=D)
                nc.scalar.dma_start(out=mk[:, 0:csz, :],
                                     in_=sm_v[:, tb - XG:tb - XG + csz, :])
                pr = pred_sb[:, tb:tb + csz, :]
                nc.vector.tensor_tensor(out=pr, in0=pr, in1=mk[:, 0:csz, :],
                                        op=OP.mult)
                # masked rows are one-nonzero-among-zeros: bf16 pairwise
                # fold is exact and runs at 2x; the 1x reduce sees half
                nc.vector.tensor_tensor(
                    out=pr[:, :, 0:D // 2], in0=pr[:, :, 0:D // 2],
                    in1=pr[:, :, D // 2:D], op=OP.add)
                nc.vector.tensor_tensor(
                    out=pr[:, :, 0:D // 4], in0=pr[:, :, 0:D // 4],
                    in1=pr[:, :, D // 4:D // 2], op=OP.add)
                nc.vector.tensor_tensor(
                    out=pr[:, :, 0:D // 8], in0=pr[:, :, 0:D // 8],
                    in1=pr[:, :, D // 8:D // 4], op=OP.add)
                nc.vector.tensor_reduce(
                    out=scores[:, tb:tb + csz], in_=pr[:, :, 0:D // 8],
                    axis=AX.X, op=OP.add)
            else:
                for b in range(csz):
                    t = tb + b
                    nc.vector.scalar_tensor_tensor(
                        out=pred_sb[:, t, :], in0=iota,
                        scalar=labs[:, t:t + 1],
                        in1=pred_sb[:, t, :], op0=OP.is_equal, op1=OP.mult,
                        accum_out=scores[:, t:t + 1])

        # extracts AFTER all stt issues: the Vector queue is in-order, so an
        # extract waiting on a late gather must not block pending stt work.
        for tb, csz in gchunks:
            o2 = out2[:, tb * 16:(tb + csz) * 16, :]
            nc.vector.tensor_tensor(
                out=o2, in0=o2,
                in1=msks[:, tb:tb + csz, :].rearrange(
                    "p t (a b) -> p (t a) b", b=2),
                op=OP.mult)
            o3 = out2[:, tb * 16:(tb + csz) * 16, :].rearrange(
                "p (t a) b -> p t (a b)", a=16)
            nc.vector.tensor_tensor(
                out=o3[:, :, 0:16], in0=o3[:, :, 0:16], in1=o3[:, :, 16:32],
                op=OP.add)
            nc.vector.tensor_reduce(
                out=scores[:, tb:tb + csz], in_=o3[:, :, 0:16],
                axis=AX.X, op=OP.add)

        # --- phase 2: per partition, 4 whole groups along the free axis ---
        gpp = NTILES // GS
        pp = singles.tile([P, gpp], f32)             # pos sums per group
        negacc = singles.tile([P, 2 * gpp], f32)     # top8-sum & 9th cols
        for g in range(gpp):
            stg = scores[:, g * GS:(g + 1) * GS]
            ptmp = ph2.tile([P, SS], f32, tag="ptmp")
            nc.scalar.activation(
                out=ptmp, in_=stg[:, 0:SS], func=AF.Relu,
                bias=1.0, scale=-1.0, accum_out=pp[:, g:g + 1])
            nl = ph2.tile([P, GS - SS], f32, tag="nl")
            nc.scalar.activation(
                out=nl, in_=stg[:, SS:GS],
                func=AF.Relu, bias=1.0, scale=1.0)
            m8 = ph2.tile([P, 8], f32, tag="m8")
            nc.vector.max(out=m8, in_=nl)
            nc.vector.match_replace(
                out=nl, in_to_replace=m8, in_values=nl, imm_value=-1.0)
            s8 = ph2.tile([P, 8], f32, tag="s8")
            nc.scalar.activation(
                out=s8, in_=m8, func=AF.Relu, bias=0.0, scale=1.0,
                accum_out=negacc[:, 2 * g:2 * g + 1])
            nc.vector.tensor_reduce(
                out=negacc[:, 2 * g + 1:2 * g + 2], in_=nl, axis=AX.X,
                op=OP.max)

        # --- final per-partition reduction -> [P, 2] (on Scalar) ---
        res = singles.tile([P, 2], f32)
        fp = ph2.tile([P, gpp], f32, tag="fp")
        nc.scalar.activation(out=fp, in_=pp, func=AF.Relu, bias=0.0,
                             scale=1.0, accum_out=res[:, 0:1])
        fn = ph2.tile([P, 2 * gpp], f32, tag="fn")
        nc.scalar.activation(out=fn, in_=negacc, func=AF.Relu, bias=0.0,
                             scale=1.0, accum_out=res[:, 1:2])
        nc.sync.dma_start(out=out, in_=res)


def _get_compiled():
    global _compiled
    if _compiled is None:
        _compiled = build_nc()
    return _compiled


def _prep_core_inputs(pred, labels):
    """Split full inputs into per-core input maps."""
    import ml_dtypes
    pred = np.asarray(pred).astype(ml_dtypes.bfloat16)
    lab = np.asarray(labels).astype(np.int64)
    k16 = (np.arange(XG, dtype=np.int64) % CHUNK)[None, :]      # [1, XG]
    qsel = (np.arange(P, dtype=np.int64) % 16)                  # [P]
    in_maps = []
    for c in range(N_CORES):
        sl = slice(c * ROWS, (c + 1) * ROWS)
        lab_sh = (lab[sl] - 1).reshape(P, NTILES)                # int64
        lg = lab_sh[:, :XG]                                      # [P, XG]
        idxs = (k16 * (D // 2) + (lg >> 1)).astype(np.int16)
        # msk[p, t, q*2+e] = (q == p%16) & (e == lab%2)
        msk = np.zeros((P, XG, 32), dtype=ml_dtypes.bfloat16)
        e = (lg & 1).astype(np.int64)                            # [P, XG]
        pi = np.arange(P)[:, None]
        ti = np.arange(XG)[None, :]
        msk[pi, ti, qsel[:, None] * 2 + e] = 1
        ls = lab_sh[:, XG:]                                      # [P, XS]
        smask = np.zeros((P, XS, D), dtype=ml_dtypes.bfloat16)
        smask[np.arange(P)[:, None], np.arange(XS)[None, :], ls] = 1
        in_maps.append({
            "pred": np.ascontiguousarray(pred[sl]),
            "smask": np.ascontiguousarray(smask.reshape(P, XS * D)),
            "idx": np.ascontiguousarray(idxs),
            "msk": np.ascontiguousarray(msk.reshape(P, XG * 32)),
        })
    return in_maps


def _finalize(results):
    pos = 0.0
    neg = 0.0
    for r in results:
        part = r["partial"].astype(np.float64)
        pos += part[:, 0].sum()
        neg += part[:, 1].sum()
    num_pos = (N_FULL // GS) * SS
    num_neg = N_FULL - num_pos
    denom = float(num_pos + int(num_neg * OHEM_RATIO))
    return np.float32((pos + neg) / denom)


def kernel(pred, labels, sample_split, sample_group_size):
    assert int(sample_split) == SS and int(sample_group_size) == GS
    from concourse.bass_utils import run_bass_kernel_spmd

    nc = _get_compiled()
    in_maps = _prep_core_inputs(pred, labels)
    res = run_bass_kernel_spmd(nc, in_maps, core_ids=list(range(N_CORES)))
    return _finalize(res.results)


# revision 20
# speedup vs baseline: 1.5576x; 1.0229x over previous
"""Trainium2 Bass kernel for nn_CompletenessLoss (OHEM hinge loss with top-k).

Self-contained: accepts FULL inputs, shards over 8 NeuronCores internally
(data-parallel over the group dimension), returns the full scalar output.

Math (reference):
  scores[i]  = pred[i, labels[i]-1]
  groups of 64 rows: first 8 are "positive", last 56 are "negative"
  pos_ls = sum over all positive rows of relu(1 - s)
  neg_ls = sum over groups of (sum of top-9 of relu(1 + s) over 56 negatives)
  out    = (pos_ls + neg_ls) / (num_pos + int(num_neg * 0.17))

V2 gather strategy (per core, 32768 rows = 128 partitions x 256 rows):
  The label-indexed gather is split across two engines running in parallel:
  - rows t in [0, XG): GPSIMD ap_gather. Indices are shared per 16-partition
    group, so the host packs index lists where position 16k+q holds the
    index of partition (16g+q)'s row k; each partition's wanted value lands
    on the "diagonal" position 16k+(p%16). A static-per-input mask multiply
    + segmented reduce (DVE, cheap) extracts the diagonal.
  - rows t in [XG, 256): DVE scalar_tensor_tensor (iota==label)*pred with
    fused accumulate (the V1 path).
  Both write into one scores[P, 256] tile; phase 2 (hinge + top-9) as in V1.

  Measured constraints that pin this design (from session traces):
  - ap_gather costs ~28.5ns/index (RD_CMD serialization, ReadOverlap=0),
    independent of instruction granularity -> the gather lane saturates at
    ~80-100 rows within the DMA window.
  - DVE mask-select costs ~270ns/row incl. the one-hot mask DMA (400B/row),
    and GPSIMD shares its SBUF port with DVE (gathers run ~2x slower while
    DVE is busy), so the kernel is jointly DMA- and DVE-bound at ~98us.
"""

import numpy as np

# Problem geometry (hardcoded per the harness contract).
N_FULL = 262144
D = 200                      # pred_dim
GS = 64                      # sample_group_size
SS = 8                       # sample_split (positives per group)
OHEM_RATIO = 0.17
KEEP = int((GS - SS) * OHEM_RATIO)   # 9 hardest negatives kept per group

N_CORES = 8
ROWS = N_FULL // N_CORES     # 32768 rows per core
P = 128                      # SBUF partitions
NTILES = ROWS // P           # 256 rows per partition = 4 groups
CHUNK = 16                   # rows-per-partition per DMA/compute chunk
XG = 80                      # rows per partition gathered on GPSIMD
XS = NTILES - XG             # rows via host one-hot mask + DVE mult/reduce
XD = 0

_compiled = None             # cached program so repeat calls skip rebuild


def build_nc():
    """Build the per-core Bass program (SPMD across the 8 cores)."""
    import concourse.bacc as bacc
    import concourse.tile as tile
    from concourse import mybir

    f32 = mybir.dt.float32
    bf16 = mybir.dt.bfloat16
    i16 = mybir.dt.int16

    nc = bacc.Bacc("TRN2", target_bir_lowering=False, debug=False,
                   num_devices=N_CORES)
    pred_t = nc.dram_tensor("pred", [ROWS, D], bf16, kind="ExternalInput")
    # idx[p, t] = (t%16)*100 + lab>>1 for gpsimd rows t in [0, XG)
    idx_t = nc.dram_tensor("idx", [P, XG], i16, kind="ExternalInput")
    # msk[p, t*32 + q*2 + e] = (q == p%16) & (e == lab%2), bf16
    msk_t = nc.dram_tensor("msk", [P, XG * 32], bf16, kind="ExternalInput")
    # smask: host one-hot mask for the s-lane rows, streamed per chunk
    smask_t = nc.dram_tensor("smask", [P, XS * D], bf16, kind="ExternalInput")
    out_t = nc.dram_tensor("partial", [P, 2], f32, kind="ExternalOutput")

    with tile.TileContext(nc) as tc:
        _body(tc, pred_t.ap(), idx_t.ap(), msk_t.ap(), smask_t.ap(),
              out_t.ap())
    nc.compile()
    return nc


def _body(tc, pred, idx, msk, smask, out):
    from concourse import mybir
    import concourse.bass as bass
    from contextlib import ExitStack

    nc = tc.nc
    f32 = mybir.dt.float32
    bf16 = mybir.dt.bfloat16
    i16 = mybir.dt.int16
    AX = mybir.AxisListType
    OP = mybir.AluOpType
    AF = mybir.ActivationFunctionType

    with ExitStack() as ctx:
        singles = ctx.enter_context(tc.tile_pool(name="singles", bufs=1))
        ph2 = ctx.enter_context(tc.tile_pool(name="ph2", bufs=2))
        scr = ctx.enter_context(tc.tile_pool(name="scr", bufs=4))

        # --- warm-up FIRST, with zero DMA dependencies, so the gather
        # ucode IRAM load (~6us) and engine dispatch are paid by ~15us.
        wz_idx = singles.tile([P, 16], i16)
        nc.gpsimd.memset(wz_idx, 0)
        wdat = singles.tile([P, 16, 2], bf16)
        nc.gpsimd.memset(wdat.rearrange("p a b -> p (a b)"), 0)
        warm3 = singles.tile([P, 16, 2], bf16)
        nc.gpsimd.ap_gather(out_ap=warm3, in_ap=wdat,
                            idxs_ap=wz_idx[:, 0:1],
                            channels=P, num_elems=16, d=2, num_idxs=16)
        wv = singles.tile([P, 2], f32)
        nc.vector.memset(wv, 0.0)
        nc.vector.tensor_scalar(out=wv, in0=wv, scalar1=0.0, scalar2=1.0,
                                op0=OP.mult, op1=OP.mult)
        wa = singles.tile([P, 2], f32)
        nc.scalar.activation(out=wa, in_=wv, func=AF.Relu,
                             bias=1.0, scale=-1.0)

        # --- one-time inputs: idx first (gathers need it early); the
        # extract masks (msks) are DMA'd after the first two smask chunks
        # so lane-B's first chunk lands as early as possible.
        idxs = singles.tile([P, XG], i16)
        nc.scalar.dma_start(out=idxs, in_=idx)
        msks = singles.tile([P, XG, 32], bf16)

        pred_sb = singles.tile([P, NTILES, D], bf16)
        out2 = singles.tile([P, XG * 16, 2], bf16)
        scores = singles.tile([P, NTILES], f32)

        # --- phase 1: stream pred; gather on GPSIMD + DVE in parallel ---
        pred_v = pred.rearrange("(p t) j -> p t j", p=P)
        # gather chunks grow geometrically (amortize ~2.4us/op overhead);
        # DVE chunks stay small for pipelining. DMA order feeds both early.
        gchunks = [(i * CHUNK, CHUNK) for i in range(XG // CHUNK)]
        # s-lane uses 32-row chunks: 1.64MB DMAs sustain a higher rate
        # than 0.82MB ones and halve the DVE op count for the same bytes.
        SCH = 32
        schunks = [(XG + i * SCH, SCH) for i in range(XS // SCH)]
        if XS % SCH:
            schunks.append((XG + (XS // SCH) * SCH, XS % SCH))
        dchunks = []
        order = []
        gi, si, di = 0, 0, 0
        # 1:1 g/s interleave: the gather lane is contention-paced at
        # ~13us/chunk, so its pred chunks must all land by ~30us or the
        # chain goes data-starved at the tail (measured: g4 at 84us with
        # the old 1g:2s order).
        while gi < len(gchunks) or si < len(schunks) or di < len(dchunks):
            if gi < len(gchunks):
                order.append(("g", gchunks[gi])); gi += 1
            if si < len(schunks):
                order.append(("s", schunks[si])); si += 1
            if di < len(dchunks):
                order.append(("d", dchunks[di])); di += 1

        n_s_issued = 0
        for kind, (tb, csz) in order:
            nc.sync.dma_start(out=pred_sb[:, tb:tb + csz, :],
                              in_=pred_v[:, tb:tb + csz, :])
            if kind == "s":
                n_s_issued += 1
                if n_s_issued == 3:
                    nc.scalar.dma_start(out=msks, in_=msk)
            if kind == "g":
                # gpsimd gather: shared indices per 16-partition group
                nc.gpsimd.ap_gather(
                    out_ap=out2[:, tb * 16:(tb + csz) * 16, :],
                    in_ap=pred_sb[:, tb:tb + csz, :].rearrange(
                        "p t (a b) -> p (t a) b", b=2),
                    idxs_ap=idxs[:, tb:tb + csz],
                    channels=P, num_elems=csz * (D // 2), d=2,
                    num_idxs=csz * 16)
            elif kind == "s":
                # host one-hot mask chunk arrives on the scalar DMA queue;
                # DVE does bulk 2x multiply (in place over pred) + seg reduce
                mk = scr.tile([P, 32, D], bf16, tag="mk")
                sm_v = smask.rearrange("p (t j) -> p t j", j=D)
                nc.scalar.dma_start(out=mk[:, 0:csz, :],
                                     in_=sm_v[:, tb - XG:tb - XG + csz, :])
                pr = pred_sb[:, tb:tb + csz, :]
                nc.vector.tensor_tensor(out=pr, in0=pr, in1=mk[:, 0:csz, :],
                                        op=OP.mult)
                # masked rows are one-nonzero-among-zeros: bf16 pairwise
                # fold is exact and runs at 2x; the 1x reduce sees half
                nc.vector.tensor_tensor(
                    out=pr[:, :, 0:D // 2], in0=pr[:, :, 0:D // 2],
                    in1=pr[:, :, D // 2:D], op=OP.add)
                nc.vector.tensor_tensor(
                    out=pr[:, :, 0:D // 4], in0=pr[:, :, 0:D // 4],
                    in1=pr[:, :, D // 4:D // 2], op=OP.add)
                nc.vector.tensor_tensor(
                    out=pr[:, :, 0:D // 8], in0=pr[:, :, 0:D // 8],
                    in1=pr[:, :, D // 8:D // 4], op=OP.add)
                nc.vector.tensor_reduce(
                    out=scores[:, tb:tb + csz], in_=pr[:, :, 0:D // 8],
                    axis=AX.X, op=OP.add)
            else:
                for b in range(csz):
                    t = tb + b
                    nc.vector.scalar_tensor_tensor(
                        out=pred_sb[:, t, :], in0=iota,
                        scalar=labs[:, t:t + 1],
                        in1=pred_sb[:, t, :], op0=OP.is_equal, op1=OP.mult,
                        accum_out=scores[:, t:t + 1])

        # extracts AFTER all stt issues: the Vector queue is in-order, so an
        # extract waiting on a late gather must not block pending stt work.
        for tb, csz in gchunks:
            o2 = out2[:, tb * 16:(tb + csz) * 16, :]
            nc.vector.tensor_tensor(
                out=o2, in0=o2,
                in1=msks[:, tb:tb + csz, :].rearrange(
                    "p t (a b) -> p (t a) b", b=2),
                op=OP.mult)
            o3 = out2[:, tb * 16:(tb + csz) * 16, :].rearrange(
                "p (t a) b -> p t (a b)", a=16)
            nc.vector.tensor_tensor(
                out=o3[:, :, 0:16], in0=o3[:, :, 0:16], in1=o3[:, :, 16:32],
                op=OP.add)
            nc.vector.tensor_reduce(
                out=scores[:, tb:tb + csz], in_=o3[:, :, 0:16],
                axis=AX.X, op=OP.add)

        # --- phase 2: per partition, 4 whole groups along the free axis ---
        gpp = NTILES // GS
        pp = singles.tile([P, gpp], f32)             # pos sums per group
        negacc = singles.tile([P, 2 * gpp], f32)     # top8-sum & 9th cols
        for g in range(gpp):
            stg = scores[:, g * GS:(g + 1) * GS]
            ptmp = ph2.tile([P, SS], f32, tag="ptmp")
            nc.scalar.activation(
                out=ptmp, in_=stg[:, 0:SS], func=AF.Relu,
                bias=1.0, scale=-1.0, accum_out=pp[:, g:g + 1])
            nl = ph2.tile([P, GS - SS], f32, tag="nl")
            nc.scalar.activation(
                out=nl, in_=stg[:, SS:GS],
                func=AF.Relu, bias=1.0, scale=1.0)
            m8 = ph2.tile([P, 8], f32, tag="m8")
            nc.vector.max(out=m8, in_=nl)
            nc.vector.match_replace(
                out=nl, in_to_replace=m8, in_values=nl, imm_value=-1.0)
            s8 = ph2.tile([P, 8], f32, tag="s8")
            nc.scalar.activation(
                out=s8, in_=m8, func=AF.Relu, bias=0.0, scale=1.0,
                accum_out=negacc[:, 2 * g:2 * g + 1])
            nc.vector.tensor_reduce(
                out=negacc[:, 2 * g + 1:2 * g + 2], in_=nl, axis=AX.X,
                op=OP.max)

        # --- final per-partition reduction -> [P, 2] (on Scalar) ---
        res = singles.tile([P, 2], f32)
        fp = ph2.tile([P, gpp], f32, tag="fp")
        nc.scalar.activation(out=fp, in_=pp, func=AF.Relu, bias=0.0,
                             scale=1.0, accum_out=res[:, 0:1])
        fn = ph2.tile([P, 2 * gpp], f32, tag="fn")
        nc.scalar.activation(out=fn, in_=negacc, func=AF.Relu, bias=0.0,
                             scale=1.0, accum_out=res[:, 1:2])
        nc.sync.dma_start(out=out, in_=res)


def _get_compiled():
    global _compiled
    if _compiled is None:
        _compiled = build_nc()
    return _compiled


def _prep_core_inputs(pred, labels):
    """Split full inputs into per-core input maps."""
    import ml_dtypes
    pred = np.asarray(pred).astype(ml_dtypes.bfloat16)
    lab = np.asarray(labels).astype(np.int64)
    k16 = (np.arange(XG, dtype=np.int64) % CHUNK)[None, :]      # [1, XG]
    qsel = (np.arange(P, dtype=np.int64) % 16)                  # [P]
    in_maps = []
    for c in range(N_CORES):
        sl = slice(c * ROWS, (c + 1) * ROWS)
        lab_sh = (lab[sl] - 1).reshape(P, NTILES)                # int64
        lg = lab_sh[:, :XG]                                      # [P, XG]
        idxs = (k16 * (D // 2) + (lg >> 1)).astype(np.int16)
        # msk[p, t, q*2+e] = (q == p%16) & (e == lab%2)
        msk = np.zeros((P, XG, 32), dtype=ml_dtypes.bfloat16)
        e = (lg & 1).astype(np.int64)                            # [P, XG]
        pi = np.arange(P)[:, None]
        ti = np.arange(XG)[None, :]
        msk[pi, ti, qsel[:, None] * 2 + e] = 1
        ls = lab_sh[:, XG:]                                      # [P, XS]
        smask = np.zeros((P, XS, D), dtype=ml_dtypes.bfloat16)
        smask[np.arange(P)[:, None], np.arange(XS)[None, :], ls] = 1
        in_maps.append({
            "pred": np.ascontiguousarray(pred[sl]),
            "smask": np.ascontiguousarray(smask.reshape(P, XS * D)),
            "idx": np.ascontiguousarray(idxs),
            "msk": np.ascontiguousarray(msk.reshape(P, XG * 32)),
        })
    return in_maps


def _finalize(results):
    pos = 0.0
    neg = 0.0
    for r in results:
        part = r["partial"].astype(np.float64)
        pos += part[:, 0].sum()
        neg += part[:, 1].sum()
    num_pos = (N_FULL // GS) * SS
    num_neg = N_FULL - num_pos
    denom = float(num_pos + int(num_neg * OHEM_RATIO))
    return np.float32((pos + neg) / denom)


def kernel(pred, labels, sample_split, sample_group_size):
    assert int(sample_split) == SS and int(sample_group_size) == GS
    from concourse.bass_utils import run_bass_kernel_spmd

    nc = _get_compiled()
    in_maps = _prep_core_inputs(pred, labels)
    res = run_bass_kernel_spmd(nc, in_maps, core_ids=list(range(N_CORES)))
    return _finalize(res.results)
